# revision 38
# baseline (speedup 1.0000x reference)
"""LRU forward kernel for Trainium2 (8 NeuronCores, batch-parallel).

Strategy (per core = one batch element):
  x_t = Lam x_{t-1} + gamma*(B u_t);  y_t = Re(C x_t) + D u_t
  with Lam = r e^{i theta} diagonal. Chunk-local rotation: within a
  512-step chunk, substituting x_{b+tau} = e^{i theta tau} z_tau
  decouples the complex recurrence into two REAL scans
      z_tau = r z_{tau-1} + w_tau,   w_tau = e^{-i theta tau} (gamma B u)
  mapped onto DVE tensor_tensor_scan. The chunk-local tables
  cos(theta tau)/sin(theta tau) (tau in [0,512)) are SBUF-resident and
  shared by every chunk; crossing a chunk boundary rotates the carry by
  e^{i 512 theta} with per-partition scalars. All matmul operands and
  DVE elementwise ops are bf16 (PE runs 1 cycle/row either way; DVE
  gets 2x mode; DMA bytes halve); scans and carries stay fp32.
  Matmuls on the PE array:
    Bu^T[n,tau] = bgt.T @ uT          (contract H)
    y[t,o]      = x^T.T @ C^T + uT.T @ D^T   (contract N / H)
  PE instruction order is software-pipelined
  (Bu0, Bu1, y0, Bu2, y1, Bu3, y2, y3) with dummy warm-up matmuls up
  front so the HAM clock-gate releases before real work arrives.
"""

import numpy as np
from contextlib import ExitStack

import concourse.bass as bass
import concourse.tile as tile
from concourse import mybir
from concourse.bass import ts
from concourse.bass_utils import run_bass_kernel_spmd

dt = mybir.dt
F32 = dt.float32
BF16 = dt.bfloat16

B, L, H, N = 8, 2048, 1024, 512
P = 128
LC = 512            # chunk length (scan + matmul moving free dim)
NLC = L // LC       # 4 chunks
NT = N // P         # 4 n-tiles
HT = H // P         # 8 h-tiles
MULT = mybir.AluOpType.mult
ADD = mybir.AluOpType.add

_COMPILED = {}


# --- workaround: walrus TPB_CTRL codegen rejects >1 sem wait on the Tile
# kernel-tail drain; spread overflow waits across trailing SP nops.
def _patched_drain_and_barrier(self, tick_clock, wait_clock):
    from concourse.tile import ScopedClock
    drain_inst = self.nc.sync.drain()
    wait_clock.add_sem_waits(drain_inst.ins,
                             ScopedClock({None: tick_clock.global_clock}))
    si = drain_inst.ins.sync_info
    if si is not None and si.on_wait and len(si.on_wait) > 1:
        waits = list(si.on_wait)
        si.on_wait = waits[:1]
        for w in waits[1:]:
            nop = self.nc.sync.nop()
            nsi = nop.ins.sync_info
            if nsi is None:
                nop.ins.sync_info = mybir.SyncInfo(on_wait=[w], on_update=[])
            else:
                nsi.on_wait = [w]
    self.nc.all_engine_barrier()
    assert self.sems is not None
    popped = self.nc._tile_sem_poison_stack.pop()
    assert popped is self._sem_poison
    self.nc.clear_and_free_semaphores(list(self.sems.allocated().values()))
    self.nc.all_engine_barrier()


tile.TileContext._drain_and_barrier = _patched_drain_and_barrier


_NOPCTR = [0]


def _split_waits(nc, cap=1):
    """Walrus setupSyncWait rejects instructions with more than `cap` sem
    waits; move overflow waits onto same-engine NoOps inserted before."""
    for f in nc.m.functions:
        for blk in f.blocks:
            out = []
            for inst in blk.instructions:
                si = inst.sync_info
                if si is not None and si.on_wait and len(si.on_wait) > cap:
                    waits = list(si.on_wait)
                    si.on_wait = waits[:cap]
                    rest = waits[cap:]
                    for i in range(0, len(rest), cap):
                        _NOPCTR[0] += 1
                        nop = mybir.InstNoOp(name=f"waitsplit-{_NOPCTR[0]}",
                                             ins=[], outs=[])
                        nop.engine = inst.engine
                        nop.sync_info = mybir.SyncInfo(
                            on_wait=rest[i:i + cap], on_update=[])
                        out.append(nop)
                out.append(inst)
            blk.instructions[:] = out


def _build():
    nc = bass.Bass()
    uT_ext = nc.declare_dram_parameter("uT", [H, L], BF16, isOutput=False)
    # bgt is n-major blocked: [H, NT blocks of [re_n | im_n] 256 cols] so a
    # single n's stationary slices are one small contiguous DMA — the first
    # Bu group then gates on 1.5 MB instead of 4 MB of preamble DMA.
    bgt_ext = nc.declare_dram_parameter("bgt", [H, 2 * N], BF16,
                                        isOutput=False)
    ctre_ext = nc.declare_dram_parameter("ctre", [N, H], BF16, isOutput=False)
    ctim_ext = nc.declare_dram_parameter("ctim", [N, H], BF16, isOutput=False)
    dtw_ext = nc.declare_dram_parameter("dtw", [H, H], BF16, isOutput=False)
    cs_ext = nc.declare_dram_parameter("cs", [N, 2 * LC], BF16,
                                       isOutput=False)
    sc_ext = nc.declare_dram_parameter("sc", [N, 2 * LC], BF16,
                                       isOutput=False)
    rb_ext = nc.declare_dram_parameter("rb", [N, LC], F32, isOutput=False)
    c3_ext = nc.declare_dram_parameter("c3", [N, 3], F32, isOutput=False)
    y_ext = nc.declare_dram_parameter("y", [L, H], F32, isOutput=True)

    with tile.TileContext(nc) as tc, ExitStack() as ctx:
        wts = ctx.enter_context(tc.tile_pool(name="wts", bufs=1))
        psum = ctx.enter_context(tc.tile_pool(name="psum", bufs=7,
                                              space="PSUM"))
        psdm = ctx.enter_context(tc.tile_pool(name="psdm", bufs=1,
                                              space="PSUM"))
        pu = ctx.enter_context(tc.tile_pool(name="pu", bufs=2))
        pv = ctx.enter_context(tc.tile_pool(name="pv", bufs=2))
        pt = ctx.enter_context(tc.tile_pool(name="pt", bufs=2))
        pw = ctx.enter_context(tc.tile_pool(name="pw", bufs=2))
        pz = ctx.enter_context(tc.tile_pool(name="pz", bufs=2))
        px = ctx.enter_context(tc.tile_pool(name="px", bufs=2))
        py_ = ctx.enter_context(tc.tile_pool(name="py", bufs=2))
        psm = ctx.enter_context(tc.tile_pool(name="psm", bufs=2))

        # ---- PE warm-up scratch ------------------------------------------
        wdum = wts.tile([P, LC], BF16, tag="wdum", name="wdum")
        nc.vector.memset(wdum[:], 0.0)

        # DMA issue is need-ordered across the three DMA-capable queues so
        # the first Bu groups gate on ~1.5 MB, not the whole 14 MB preamble.
        # scalar stays light so ACT_TABLE_LOAD + first casts run on time.
        uc = {}
        for lc in range(NLC):
            for k in range(HT):
                uc[lc, k] = pu.tile([P, LC], BF16, tag=f"u{k}",
                                    name=f"u{lc}_{k}")
        bgtn = {}
        for n in range(NT):
            for k in range(HT):
                bgtn[n, k] = wts.tile([P, 256], BF16, tag=f"bg{n}_{k}",
                                      name=f"bg{n}_{k}")
        cs, sc, rb, c3 = {}, {}, {}, {}
        for n in range(NT):
            cs[n] = wts.tile([P, 2 * LC], BF16, tag=f"cs{n}", name=f"cs{n}")
            sc[n] = wts.tile([P, 2 * LC], BF16, tag=f"sc{n}", name=f"sc{n}")
            rb[n] = wts.tile([P, LC], F32, tag=f"rb{n}", name=f"rb{n}")
            c3[n] = wts.tile([P, 3], F32, tag=f"c3{n}", name=f"c3{n}")
        ctre, ctim, dtw = {}, {}, {}
        for n in range(NT):
            ctre[n] = wts.tile([P, H], BF16, tag=f"ctre{n}", name=f"ctre{n}")
            ctim[n] = wts.tile([P, H], BF16, tag=f"ctim{n}", name=f"ctim{n}")
        for k in range(HT):
            dtw[k] = wts.tile([P, H], BF16, tag=f"dtw{k}", name=f"dtw{k}")

        def _ld_u(eng, lc, k):
            eng.dma_start(out=uc[lc, k][:],
                          in_=uT_ext[ts(k, P), lc * LC:(lc + 1) * LC])

        def _ld_bg(eng, n, k):
            eng.dma_start(out=bgtn[n, k][:],
                          in_=bgt_ext[ts(k, P), n * 256:(n + 1) * 256])

        # gpsimd: u chunk0 k0-3, n0-tables, bgt n2, rb/c3, bgt n3, rest of
        # tables (need-ordered)
        for k in range(4):
            _ld_u(nc.gpsimd, 0, k)
        nc.gpsimd.dma_start(out=cs[0][:], in_=cs_ext[ts(0, P), :])
        nc.gpsimd.dma_start(out=sc[0][:], in_=sc_ext[ts(0, P), :])
        for k in range(HT):
            _ld_bg(nc.gpsimd, 2, k)
        for k in range(HT):
            _ld_bg(nc.gpsimd, 3, k)
        nc.gpsimd.dma_start(out=rb[0][:], in_=rb_ext[ts(0, P), :])
        nc.gpsimd.dma_start(out=c3[0][:], in_=c3_ext[ts(0, P), :])
        for n in range(1, NT):
            nc.gpsimd.dma_start(out=cs[n][:], in_=cs_ext[ts(n, P), :])
            nc.gpsimd.dma_start(out=sc[n][:], in_=sc_ext[ts(n, P), :])
            nc.gpsimd.dma_start(out=rb[n][:], in_=rb_ext[ts(n, P), :])
            nc.gpsimd.dma_start(out=c3[n][:], in_=c3_ext[ts(n, P), :])
        # scalar: u chunk0 k4-7 + bgt n1, then it handles casts
        for k in range(4, HT):
            _ld_u(nc.scalar, 0, k)
        for k in range(HT):
            _ld_bg(nc.scalar, 1, k)
        # sync: bgt n0, u chunk1, later the deferred weights + y stores
        for k in range(HT):
            _ld_bg(nc.sync, 0, k)
        for k in range(HT):
            _ld_u(nc.sync, 1, k)

        # carries (persistent, fp32)
        cre, cim = {}, {}
        for n in range(NT):
            cre[n] = wts.tile([P, 1], F32, tag=f"cre{n}", name=f"cre{n}")
            cim[n] = wts.tile([P, 1], F32, tag=f"cim{n}", name=f"cim{n}")
            nc.vector.memset(cre[n][:], 0.0)
            nc.vector.memset(cim[n][:], 0.0)

        # ---- PE warm-up: dummy matmuls while the preamble DMAs land ------
        pdum = psdm.tile([P, LC], F32, tag="pdum", name="pdum")
        NDUM = 14
        for i in range(NDUM):
            nc.tensor.matmul(pdum[:], wdum[:, 0:P], wdum[:],
                             start=(i == 0), stop=(i == NDUM - 1))

        xr = {}
        xi = {}
        zstash = {}

        def emit_bu(lc):
            for n in range(NT):
                psr = psum.tile([P, LC], F32, tag="ps", name=f"psr{lc}_{n}")
                psi = psum.tile([P, LC], F32, tag="ps", name=f"psi{lc}_{n}")
                for k in range(HT):
                    nc.tensor.matmul(psr[:], bgtn[n, k][:, 0:P],
                                     uc[lc, k][:],
                                     start=(k == 0), stop=(k == HT - 1))
                for k in range(HT):
                    nc.tensor.matmul(psi[:], bgtn[n, k][:, P:2 * P],
                                     uc[lc, k][:],
                                     start=(k == 0), stop=(k == HT - 1))
                # PSUM fp32 -> SBUF bf16, stacked [vr | vi]
                vrvi = pv.tile([P, 2 * LC], BF16, tag="v", name=f"v{lc}_{n}")
                nc.scalar.copy(vrvi[:, 0:LC], psr[:])
                nc.scalar.copy(vrvi[:, LC:2 * LC], psi[:])
                # mod: w = e^{-i theta tau} * v
                pq = pt.tile([P, 2 * LC], BF16, tag="pq", name=f"p1_{lc}_{n}")
                nc.vector.tensor_mul(pq[:], cs[n][:], vrvi[:])   # [c.vr|s.vi]
                wrwi = pw.tile([P, 2 * LC], BF16, tag="w", name=f"w{lc}_{n}")
                nc.vector.tensor_add(wrwi[:, 0:LC], pq[:, 0:LC],
                                     pq[:, LC:2 * LC])
                pq2 = pt.tile([P, 2 * LC], BF16, tag="pq", name=f"p2_{lc}_{n}")
                nc.vector.tensor_mul(pq2[:], sc[n][:], vrvi[:])  # [s.vr|c.vi]
                nc.vector.tensor_sub(wrwi[:, LC:2 * LC], pq2[:, LC:2 * LC],
                                     pq2[:, 0:LC])
                # chunk scans with fp32 state/carry
                zrzi = pz.tile([P, 2 * LC], BF16, tag="z", name=f"z{lc}_{n}")
                zstash[lc, n] = zrzi
                nc.vector.tensor_tensor_scan(zrzi[:, 0:LC], rb[n][:],
                                             wrwi[:, 0:LC], cre[n][:, 0:1],
                                             MULT, ADD)
                nc.vector.tensor_tensor_scan(zrzi[:, LC:2 * LC], rb[n][:],
                                             wrwi[:, LC:2 * LC],
                                             cim[n][:, 0:1], MULT, ADD)
                # demod: x = e^{+i theta tau} * z
                pq3 = pt.tile([P, 2 * LC], BF16, tag="pq", name=f"p3_{lc}_{n}")
                nc.vector.tensor_mul(pq3[:], cs[n][:], zrzi[:])  # [c.zr|s.zi]
                xr[lc, n] = px.tile([P, LC], BF16, tag=f"xr{n}",
                                    name=f"xr{lc}_{n}")
                nc.vector.tensor_sub(xr[lc, n][:], pq3[:, 0:LC],
                                     pq3[:, LC:2 * LC])
                pq4 = pt.tile([P, 2 * LC], BF16, tag="pq", name=f"p4_{lc}_{n}")
                nc.vector.tensor_mul(pq4[:], sc[n][:], zrzi[:])  # [s.zr|c.zi]
                xi[lc, n] = px.tile([P, LC], BF16, tag=f"xi{n}",
                                    name=f"xi{lc}_{n}")
                nc.vector.tensor_add(xi[lc, n][:], pq4[:, 0:LC],
                                     pq4[:, LC:2 * LC])
                if lc < NLC - 1:
                    # carry' = e^{i 512 theta} * z[last]; emitted after demod
                    # so it never delays the x tiles the PE is waiting on
                    # (only scan(lc+1, n) consumes it, a full chunk later).
                    # c3 cols: 0 = cos(512th), 1 = sin(512th), 2 = -sin(512th)
                    t1 = psm.tile([P, 1], F32, tag="t1", name=f"t1_{lc}_{n}")
                    nc.vector.tensor_scalar_mul(t1[:], zrzi[:, LC - 1:LC],
                                                c3[n][:, 0:1])
                    nc.vector.scalar_tensor_tensor(
                        cre[n][:], zrzi[:, 2 * LC - 1:2 * LC], c3[n][:, 2:3],
                        t1[:], MULT, ADD)
                    t2 = psm.tile([P, 1], F32, tag="t2", name=f"t2_{lc}_{n}")
                    nc.vector.tensor_scalar_mul(t2[:], zrzi[:, 2 * LC - 1:2 * LC],
                                                c3[n][:, 0:1])
                    nc.vector.scalar_tensor_tensor(
                        cim[n][:], zrzi[:, LC - 1:LC], c3[n][:, 1:2],
                        t2[:], MULT, ADD)

        def emit_y(lc):
            last_chunk = (lc == NLC - 1)
            for lt in range(LC // P):
                split_store = last_chunk and lt == LC // P - 1
                ys = py_.tile([P, H], F32, tag="ys", name=f"ys{lc}_{lt}")
                for oc in range(2):
                    pyb = psum.tile([P, 512], F32, tag="ps",
                                    name=f"psy{lc}_{lt}_{oc}")
                    # u.D first (depends only on u), then x-contributions in
                    # n order — lets the PE track the DVE demod chain
                    for k in range(HT):
                        nc.tensor.matmul(pyb[:], uc[lc, k][:, ts(lt, P)],
                                         dtw[k][:, ts(oc, 512)],
                                         start=(k == 0), stop=False)
                    for n in range(NT):
                        nc.tensor.matmul(pyb[:], xr[lc, n][:, ts(lt, P)],
                                         ctre[n][:, ts(oc, 512)],
                                         start=False, stop=False)
                        nc.tensor.matmul(pyb[:], xi[lc, n][:, ts(lt, P)],
                                         ctim[n][:, ts(oc, 512)],
                                         start=False, stop=(n == NT - 1))
                    nc.scalar.copy(ys[:, ts(oc, 512)], pyb[:])
                    if split_store:
                        # tail tiles: store per-oc halves so the last DMAs
                        # overlap the last psum groups instead of trailing
                        nc.sync.dma_start(
                            out=y_ext[ts(lc * (LC // P) + lt, P),
                                      ts(oc, 512)],
                            in_=ys[:, ts(oc, 512)])
                if not split_store:
                    nc.sync.dma_start(
                        out=y_ext[ts(lc * (LC // P) + lt, P), :], in_=ys[:])

        # software-pipelined emission: PE stream has no demod-wait bubbles
        emit_bu(0)
        # Gate the non-urgent weight loads behind chunk-0 progress: a tiny
        # WAW write on ctre[0] makes its DMA (and everything queued after it
        # on sync) wait until ~when the startup-critical 3.5 MB has landed,
        # so those loads stop stealing HBM bandwidth from the Bu(0) gates.
        nc.scalar.copy(ctre[0][:, 0:1], zstash[0, 0][:, 0:1])
        for n in range(NT):
            nc.sync.dma_start(out=ctre[n][:], in_=ctre_ext[ts(n, P), :])
            nc.sync.dma_start(out=ctim[n][:], in_=ctim_ext[ts(n, P), :])
        for k in range(HT):
            nc.sync.dma_start(out=dtw[k][:], in_=dtw_ext[ts(k, P), :])
        for k in range(HT):
            _ld_u(nc.sync, 2, k)
        for k in range(HT):
            _ld_u(nc.sync, 3, k)
        emit_bu(1)
        emit_y(0)
        emit_bu(2)
        emit_y(1)
        emit_bu(3)
        emit_y(2)
        emit_y(3)
    _split_waits(nc)
    return nc


def _prep(u, nu_log, theta_log, gamma_log, B_re, B_im, C_re, C_im, D):
    BF = dt.np(BF16)
    f64 = np.float64
    r = np.exp(-np.exp(nu_log.astype(f64)))
    theta = np.exp(theta_log.astype(f64))
    gamma = np.exp(gamma_log.astype(f64))
    tau = np.arange(LC, dtype=f64)
    ang = theta[:, None] * tau[None, :]
    cosv, sinv = np.cos(ang), np.sin(ang)
    cs = np.ascontiguousarray(np.concatenate([cosv, sinv], 1).astype(BF))
    sc = np.ascontiguousarray(np.concatenate([sinv, cosv], 1).astype(BF))
    rb = np.broadcast_to(r.astype(np.float32)[:, None], (N, LC)).copy()
    ang512 = (theta * LC) % (2 * np.pi)
    c3 = np.ascontiguousarray(
        np.stack([np.cos(ang512), np.sin(ang512), -np.sin(ang512)],
                 1).astype(np.float32))
    bre = (gamma[:, None] * B_re).T.reshape(H, NT, P)   # [H, n, j]
    bim = (gamma[:, None] * B_im).T.reshape(H, NT, P)
    bgt = np.ascontiguousarray(
        np.concatenate([bre, bim], 2).reshape(H, 2 * N).astype(BF))
    ctre = np.ascontiguousarray(C_re.T.astype(BF))
    ctim = np.ascontiguousarray((-C_im).T.astype(BF))
    dtw = np.ascontiguousarray(D.T.astype(BF))
    common = dict(bgt=bgt, ctre=ctre, ctim=ctim, dtw=dtw, cs=cs, sc=sc,
                  rb=rb, c3=c3)
    in_maps = []
    for b in range(B):
        m = dict(common)
        m["uT"] = np.ascontiguousarray(u[b].T.astype(BF))
        in_maps.append(m)
    return in_maps


def kernel(u, nu_log, theta_log, gamma_log, B_re, B_im, C_re, C_im, D,
           _trace=False, _tmpdir=None):
    if "nc" not in _COMPILED:
        _COMPILED["nc"] = _build()
    nc = _COMPILED["nc"]
    in_maps = _prep(u, nu_log, theta_log, gamma_log, B_re, B_im, C_re, C_im, D)
    try:
        res = run_bass_kernel_spmd(nc, in_maps, list(range(B)), trace=_trace,
                                   tmpdir=_tmpdir)
    except ModuleNotFoundError:
        res = run_bass_kernel_spmd(nc, in_maps, list(range(B)), trace=False)
    y = np.stack([res.results[i]["y"] for i in range(B)])
    kernel.last_exec_time_ns = res.exec_time_ns
    return y


# revision 39
# speedup vs baseline: 1.0064x; 1.0064x over previous
"""LRU forward kernel for Trainium2 (8 NeuronCores, batch-parallel).

Strategy (per core = one batch element):
  x_t = Lam x_{t-1} + gamma*(B u_t);  y_t = Re(C x_t) + D u_t
  with Lam = r e^{i theta} diagonal. Chunk-local rotation: within a
  512-step chunk, substituting x_{b+tau} = e^{i theta tau} z_tau
  decouples the complex recurrence into two REAL scans
      z_tau = r z_{tau-1} + w_tau,   w_tau = e^{-i theta tau} (gamma B u)
  mapped onto DVE tensor_tensor_scan. The chunk-local tables
  cos(theta tau)/sin(theta tau) (tau in [0,512)) are SBUF-resident and
  shared by every chunk; crossing a chunk boundary rotates the carry by
  e^{i 512 theta} with per-partition scalars. All matmul operands and
  DVE elementwise ops are bf16 (PE runs 1 cycle/row either way; DVE
  gets 2x mode; DMA bytes halve); scans and carries stay fp32.
  Matmuls on the PE array:
    Bu^T[n,tau] = bgt.T @ uT          (contract H)
    y[t,o]      = x^T.T @ C^T + uT.T @ D^T   (contract N / H)
  PE instruction order is software-pipelined
  (Bu0, Bu1, y0, Bu2, y1, Bu3, y2, y3) with dummy warm-up matmuls up
  front so the HAM clock-gate releases before real work arrives.
"""

import numpy as np
from contextlib import ExitStack

import concourse.bass as bass
import concourse.tile as tile
from concourse import mybir
from concourse.bass import ts
from concourse.bass_utils import run_bass_kernel_spmd

dt = mybir.dt
F32 = dt.float32
BF16 = dt.bfloat16

B, L, H, N = 8, 2048, 1024, 512
P = 128
LC = 512            # chunk length (scan + matmul moving free dim)
NLC = L // LC       # 4 chunks
NT = N // P         # 4 n-tiles
HT = H // P         # 8 h-tiles
MULT = mybir.AluOpType.mult
ADD = mybir.AluOpType.add

_COMPILED = {}


# --- workaround: walrus TPB_CTRL codegen rejects >1 sem wait on the Tile
# kernel-tail drain; spread overflow waits across trailing SP nops.
def _patched_drain_and_barrier(self, tick_clock, wait_clock):
    from concourse.tile import ScopedClock
    drain_inst = self.nc.sync.drain()
    wait_clock.add_sem_waits(drain_inst.ins,
                             ScopedClock({None: tick_clock.global_clock}))
    si = drain_inst.ins.sync_info
    if si is not None and si.on_wait and len(si.on_wait) > 1:
        waits = list(si.on_wait)
        si.on_wait = waits[:1]
        for w in waits[1:]:
            nop = self.nc.sync.nop()
            nsi = nop.ins.sync_info
            if nsi is None:
                nop.ins.sync_info = mybir.SyncInfo(on_wait=[w], on_update=[])
            else:
                nsi.on_wait = [w]
    self.nc.all_engine_barrier()
    assert self.sems is not None
    popped = self.nc._tile_sem_poison_stack.pop()
    assert popped is self._sem_poison
    self.nc.clear_and_free_semaphores(list(self.sems.allocated().values()))
    self.nc.all_engine_barrier()


tile.TileContext._drain_and_barrier = _patched_drain_and_barrier


_NOPCTR = [0]


def _split_waits(nc, cap=1):
    """Walrus setupSyncWait rejects instructions with more than `cap` sem
    waits; move overflow waits onto same-engine NoOps inserted before."""
    for f in nc.m.functions:
        for blk in f.blocks:
            out = []
            for inst in blk.instructions:
                si = inst.sync_info
                if si is not None and si.on_wait and len(si.on_wait) > cap:
                    waits = list(si.on_wait)
                    si.on_wait = waits[:cap]
                    rest = waits[cap:]
                    for i in range(0, len(rest), cap):
                        _NOPCTR[0] += 1
                        nop = mybir.InstNoOp(name=f"waitsplit-{_NOPCTR[0]}",
                                             ins=[], outs=[])
                        nop.engine = inst.engine
                        nop.sync_info = mybir.SyncInfo(
                            on_wait=rest[i:i + cap], on_update=[])
                        out.append(nop)
                out.append(inst)
            blk.instructions[:] = out


def _build():
    nc = bass.Bass()
    uT_ext = nc.declare_dram_parameter("uT", [H, L], BF16, isOutput=False)
    # bgt is n-major blocked: [H, NT blocks of [re_n | im_n] 256 cols] so a
    # single n's stationary slices are one small contiguous DMA — the first
    # Bu group then gates on 1.5 MB instead of 4 MB of preamble DMA.
    bgt_ext = nc.declare_dram_parameter("bgt", [H, 2 * N], BF16,
                                        isOutput=False)
    ctre_ext = nc.declare_dram_parameter("ctre", [N, H], BF16, isOutput=False)
    ctim_ext = nc.declare_dram_parameter("ctim", [N, H], BF16, isOutput=False)
    dtw_ext = nc.declare_dram_parameter("dtw", [H, H], BF16, isOutput=False)
    cs_ext = nc.declare_dram_parameter("cs", [N, 2 * LC], BF16,
                                       isOutput=False)
    sc_ext = nc.declare_dram_parameter("sc", [N, 2 * LC], BF16,
                                       isOutput=False)
    rb_ext = nc.declare_dram_parameter("rb", [N, LC], F32, isOutput=False)
    c3_ext = nc.declare_dram_parameter("c3", [N, 3], F32, isOutput=False)
    y_ext = nc.declare_dram_parameter("y", [L, H], F32, isOutput=True)

    with tile.TileContext(nc) as tc, ExitStack() as ctx:
        wts = ctx.enter_context(tc.tile_pool(name="wts", bufs=1))
        psum = ctx.enter_context(tc.tile_pool(name="psum", bufs=7,
                                              space="PSUM"))
        psdm = ctx.enter_context(tc.tile_pool(name="psdm", bufs=1,
                                              space="PSUM"))
        pu = ctx.enter_context(tc.tile_pool(name="pu", bufs=2))
        pv = ctx.enter_context(tc.tile_pool(name="pv", bufs=2))
        pt = ctx.enter_context(tc.tile_pool(name="pt", bufs=2))
        pw = ctx.enter_context(tc.tile_pool(name="pw", bufs=2))
        pz = ctx.enter_context(tc.tile_pool(name="pz", bufs=2))
        px = ctx.enter_context(tc.tile_pool(name="px", bufs=2))
        py_ = ctx.enter_context(tc.tile_pool(name="py", bufs=2))
        psm = ctx.enter_context(tc.tile_pool(name="psm", bufs=2))

        # ---- PE warm-up scratch ------------------------------------------
        wdum = wts.tile([P, LC], BF16, tag="wdum", name="wdum")
        nc.vector.memset(wdum[:], 0.0)

        # DMA issue is need-ordered across the three DMA-capable queues so
        # the first Bu groups gate on ~1.5 MB, not the whole 14 MB preamble.
        # scalar stays light so ACT_TABLE_LOAD + first casts run on time.
        uc = {}
        for lc in range(NLC):
            for k in range(HT):
                uc[lc, k] = pu.tile([P, LC], BF16, tag=f"u{k}",
                                    name=f"u{lc}_{k}")
        bgtn = {}
        for n in range(NT):
            for k in range(HT):
                bgtn[n, k] = wts.tile([P, 256], BF16, tag=f"bg{n}_{k}",
                                      name=f"bg{n}_{k}")
        cs, sc, rb, c3 = {}, {}, {}, {}
        for n in range(NT):
            cs[n] = wts.tile([P, 2 * LC], BF16, tag=f"cs{n}", name=f"cs{n}")
            sc[n] = wts.tile([P, 2 * LC], BF16, tag=f"sc{n}", name=f"sc{n}")
            rb[n] = wts.tile([P, LC], F32, tag=f"rb{n}", name=f"rb{n}")
            c3[n] = wts.tile([P, 3], F32, tag=f"c3{n}", name=f"c3{n}")
        ctre, ctim, dtw = {}, {}, {}
        for n in range(NT):
            ctre[n] = wts.tile([P, H], BF16, tag=f"ctre{n}", name=f"ctre{n}")
            ctim[n] = wts.tile([P, H], BF16, tag=f"ctim{n}", name=f"ctim{n}")
        for k in range(HT):
            dtw[k] = wts.tile([P, H], BF16, tag=f"dtw{k}", name=f"dtw{k}")

        def _ld_u(eng, lc, k):
            eng.dma_start(out=uc[lc, k][:],
                          in_=uT_ext[ts(k, P), lc * LC:(lc + 1) * LC])

        def _ld_bg(eng, n, k):
            eng.dma_start(out=bgtn[n, k][:],
                          in_=bgt_ext[ts(k, P), n * 256:(n + 1) * 256])

        # gpsimd: u chunk0 k0-3, n0-tables, bgt n2, rb/c3, bgt n3, rest of
        # tables (need-ordered)
        for k in range(4):
            _ld_u(nc.gpsimd, 0, k)
        nc.gpsimd.dma_start(out=cs[0][:], in_=cs_ext[ts(0, P), :])
        nc.gpsimd.dma_start(out=sc[0][:], in_=sc_ext[ts(0, P), :])
        for k in range(HT):
            _ld_bg(nc.gpsimd, 2, k)
        nc.gpsimd.dma_start(out=rb[0][:], in_=rb_ext[ts(0, P), :])
        nc.gpsimd.dma_start(out=c3[0][:], in_=c3_ext[ts(0, P), :])
        for k in range(HT):
            _ld_bg(nc.gpsimd, 3, k)
        for n in range(1, NT):
            nc.gpsimd.dma_start(out=cs[n][:], in_=cs_ext[ts(n, P), :])
            nc.gpsimd.dma_start(out=sc[n][:], in_=sc_ext[ts(n, P), :])
            nc.gpsimd.dma_start(out=rb[n][:], in_=rb_ext[ts(n, P), :])
            nc.gpsimd.dma_start(out=c3[n][:], in_=c3_ext[ts(n, P), :])
        # scalar: u chunk0 k4-7 + bgt n1, then it handles casts
        for k in range(4, HT):
            _ld_u(nc.scalar, 0, k)
        for k in range(HT):
            _ld_bg(nc.scalar, 1, k)
        # sync: bgt n0, u chunk1, later the deferred weights + y stores
        for k in range(HT):
            _ld_bg(nc.sync, 0, k)
        for k in range(HT):
            _ld_u(nc.sync, 1, k)

        # carries (persistent, fp32)
        cre, cim = {}, {}
        for n in range(NT):
            cre[n] = wts.tile([P, 1], F32, tag=f"cre{n}", name=f"cre{n}")
            cim[n] = wts.tile([P, 1], F32, tag=f"cim{n}", name=f"cim{n}")
            nc.vector.memset(cre[n][:], 0.0)
            nc.vector.memset(cim[n][:], 0.0)

        # ---- PE warm-up: dummy matmuls while the preamble DMAs land ------
        pdum = psdm.tile([P, LC], F32, tag="pdum", name="pdum")
        NDUM = 14
        for i in range(NDUM):
            nc.tensor.matmul(pdum[:], wdum[:, 0:P], wdum[:],
                             start=(i == 0), stop=(i == NDUM - 1))

        xr = {}
        xi = {}
        zstash = {}

        def emit_bu(lc):
            for n in range(NT):
                psr = psum.tile([P, LC], F32, tag="ps", name=f"psr{lc}_{n}")
                psi = psum.tile([P, LC], F32, tag="ps", name=f"psi{lc}_{n}")
                for k in range(HT):
                    nc.tensor.matmul(psr[:], bgtn[n, k][:, 0:P],
                                     uc[lc, k][:],
                                     start=(k == 0), stop=(k == HT - 1))
                for k in range(HT):
                    nc.tensor.matmul(psi[:], bgtn[n, k][:, P:2 * P],
                                     uc[lc, k][:],
                                     start=(k == 0), stop=(k == HT - 1))
                # PSUM fp32 -> SBUF bf16, stacked [vr | vi]
                vrvi = pv.tile([P, 2 * LC], BF16, tag="v", name=f"v{lc}_{n}")
                nc.scalar.copy(vrvi[:, 0:LC], psr[:])
                nc.scalar.copy(vrvi[:, LC:2 * LC], psi[:])
                # mod: w = e^{-i theta tau} * v
                pq = pt.tile([P, 2 * LC], BF16, tag="pq", name=f"p1_{lc}_{n}")
                nc.vector.tensor_mul(pq[:], cs[n][:], vrvi[:])   # [c.vr|s.vi]
                wrwi = pw.tile([P, 2 * LC], BF16, tag="w", name=f"w{lc}_{n}")
                nc.vector.tensor_add(wrwi[:, 0:LC], pq[:, 0:LC],
                                     pq[:, LC:2 * LC])
                pq2 = pt.tile([P, 2 * LC], BF16, tag="pq", name=f"p2_{lc}_{n}")
                nc.vector.tensor_mul(pq2[:], sc[n][:], vrvi[:])  # [s.vr|c.vi]
                nc.vector.tensor_sub(wrwi[:, LC:2 * LC], pq2[:, LC:2 * LC],
                                     pq2[:, 0:LC])
                # chunk scans with fp32 state/carry
                zrzi = pz.tile([P, 2 * LC], BF16, tag="z", name=f"z{lc}_{n}")
                zstash[lc, n] = zrzi
                nc.vector.tensor_tensor_scan(zrzi[:, 0:LC], rb[n][:],
                                             wrwi[:, 0:LC], cre[n][:, 0:1],
                                             MULT, ADD)
                nc.vector.tensor_tensor_scan(zrzi[:, LC:2 * LC], rb[n][:],
                                             wrwi[:, LC:2 * LC],
                                             cim[n][:, 0:1], MULT, ADD)
                # demod: x = e^{+i theta tau} * z
                pq3 = pt.tile([P, 2 * LC], BF16, tag="pq", name=f"p3_{lc}_{n}")
                nc.vector.tensor_mul(pq3[:], cs[n][:], zrzi[:])  # [c.zr|s.zi]
                xr[lc, n] = px.tile([P, LC], BF16, tag=f"xr{n}",
                                    name=f"xr{lc}_{n}")
                nc.vector.tensor_sub(xr[lc, n][:], pq3[:, 0:LC],
                                     pq3[:, LC:2 * LC])
                pq4 = pt.tile([P, 2 * LC], BF16, tag="pq", name=f"p4_{lc}_{n}")
                nc.vector.tensor_mul(pq4[:], sc[n][:], zrzi[:])  # [s.zr|c.zi]
                xi[lc, n] = px.tile([P, LC], BF16, tag=f"xi{n}",
                                    name=f"xi{lc}_{n}")
                nc.vector.tensor_add(xi[lc, n][:], pq4[:, 0:LC],
                                     pq4[:, LC:2 * LC])
                if lc < NLC - 1:
                    # carry' = e^{i 512 theta} * z[last]; emitted after demod
                    # so it never delays the x tiles the PE is waiting on
                    # (only scan(lc+1, n) consumes it, a full chunk later).
                    # c3 cols: 0 = cos(512th), 1 = sin(512th), 2 = -sin(512th)
                    t1 = psm.tile([P, 1], F32, tag="t1", name=f"t1_{lc}_{n}")
                    nc.vector.tensor_scalar_mul(t1[:], zrzi[:, LC - 1:LC],
                                                c3[n][:, 0:1])
                    nc.vector.scalar_tensor_tensor(
                        cre[n][:], zrzi[:, 2 * LC - 1:2 * LC], c3[n][:, 2:3],
                        t1[:], MULT, ADD)
                    t2 = psm.tile([P, 1], F32, tag="t2", name=f"t2_{lc}_{n}")
                    nc.vector.tensor_scalar_mul(t2[:], zrzi[:, 2 * LC - 1:2 * LC],
                                                c3[n][:, 0:1])
                    nc.vector.scalar_tensor_tensor(
                        cim[n][:], zrzi[:, LC - 1:LC], c3[n][:, 1:2],
                        t2[:], MULT, ADD)

        def emit_y(lc):
            last_chunk = (lc == NLC - 1)
            for lt in range(LC // P):
                split_store = last_chunk and lt == LC // P - 1
                ys = py_.tile([P, H], F32, tag="ys", name=f"ys{lc}_{lt}")
                for oc in range(2):
                    pyb = psum.tile([P, 512], F32, tag="ps",
                                    name=f"psy{lc}_{lt}_{oc}")
                    # u.D first (depends only on u), then x-contributions in
                    # n order — lets the PE track the DVE demod chain
                    for k in range(HT):
                        nc.tensor.matmul(pyb[:], uc[lc, k][:, ts(lt, P)],
                                         dtw[k][:, ts(oc, 512)],
                                         start=(k == 0), stop=False)
                    for n in range(NT):
                        nc.tensor.matmul(pyb[:], xr[lc, n][:, ts(lt, P)],
                                         ctre[n][:, ts(oc, 512)],
                                         start=False, stop=False)
                        nc.tensor.matmul(pyb[:], xi[lc, n][:, ts(lt, P)],
                                         ctim[n][:, ts(oc, 512)],
                                         start=False, stop=(n == NT - 1))
                    nc.scalar.copy(ys[:, ts(oc, 512)], pyb[:])
                    if split_store:
                        # tail tiles: store per-oc halves so the last DMAs
                        # overlap the last psum groups instead of trailing
                        nc.sync.dma_start(
                            out=y_ext[ts(lc * (LC // P) + lt, P),
                                      ts(oc, 512)],
                            in_=ys[:, ts(oc, 512)])
                if not split_store:
                    nc.sync.dma_start(
                        out=y_ext[ts(lc * (LC // P) + lt, P), :], in_=ys[:])

        # software-pipelined emission: PE stream has no demod-wait bubbles
        emit_bu(0)
        # Gate the non-urgent weight loads behind chunk-0 progress: a tiny
        # WAW write on ctre[0] makes its DMA (and everything queued after it
        # on sync) wait until ~when the startup-critical 3.5 MB has landed,
        # so those loads stop stealing HBM bandwidth from the Bu(0) gates.
        nc.scalar.copy(ctre[0][:, 0:1], zstash[0, 0][:, 0:1])
        for n in range(NT):
            nc.sync.dma_start(out=ctre[n][:], in_=ctre_ext[ts(n, P), :])
            nc.sync.dma_start(out=ctim[n][:], in_=ctim_ext[ts(n, P), :])
        for k in range(HT):
            nc.sync.dma_start(out=dtw[k][:], in_=dtw_ext[ts(k, P), :])
        for k in range(HT):
            _ld_u(nc.sync, 2, k)
        for k in range(HT):
            _ld_u(nc.sync, 3, k)
        emit_bu(1)
        emit_y(0)
        emit_bu(2)
        emit_y(1)
        emit_bu(3)
        emit_y(2)
        emit_y(3)
    _split_waits(nc)
    return nc


def _prep(u, nu_log, theta_log, gamma_log, B_re, B_im, C_re, C_im, D):
    BF = dt.np(BF16)
    f64 = np.float64
    r = np.exp(-np.exp(nu_log.astype(f64)))
    theta = np.exp(theta_log.astype(f64))
    gamma = np.exp(gamma_log.astype(f64))
    tau = np.arange(LC, dtype=f64)
    ang = theta[:, None] * tau[None, :]
    cosv, sinv = np.cos(ang), np.sin(ang)
    cs = np.ascontiguousarray(np.concatenate([cosv, sinv], 1).astype(BF))
    sc = np.ascontiguousarray(np.concatenate([sinv, cosv], 1).astype(BF))
    rb = np.broadcast_to(r.astype(np.float32)[:, None], (N, LC)).copy()
    ang512 = (theta * LC) % (2 * np.pi)
    c3 = np.ascontiguousarray(
        np.stack([np.cos(ang512), np.sin(ang512), -np.sin(ang512)],
                 1).astype(np.float32))
    bre = (gamma[:, None] * B_re).T.reshape(H, NT, P)   # [H, n, j]
    bim = (gamma[:, None] * B_im).T.reshape(H, NT, P)
    bgt = np.ascontiguousarray(
        np.concatenate([bre, bim], 2).reshape(H, 2 * N).astype(BF))
    ctre = np.ascontiguousarray(C_re.T.astype(BF))
    ctim = np.ascontiguousarray((-C_im).T.astype(BF))
    dtw = np.ascontiguousarray(D.T.astype(BF))
    common = dict(bgt=bgt, ctre=ctre, ctim=ctim, dtw=dtw, cs=cs, sc=sc,
                  rb=rb, c3=c3)
    in_maps = []
    for b in range(B):
        m = dict(common)
        m["uT"] = np.ascontiguousarray(u[b].T.astype(BF))
        in_maps.append(m)
    return in_maps


def kernel(u, nu_log, theta_log, gamma_log, B_re, B_im, C_re, C_im, D,
           _trace=False, _tmpdir=None):
    if "nc" not in _COMPILED:
        _COMPILED["nc"] = _build()
    nc = _COMPILED["nc"]
    in_maps = _prep(u, nu_log, theta_log, gamma_log, B_re, B_im, C_re, C_im, D)
    try:
        res = run_bass_kernel_spmd(nc, in_maps, list(range(B)), trace=_trace,
                                   tmpdir=_tmpdir)
    except ModuleNotFoundError:
        res = run_bass_kernel_spmd(nc, in_maps, list(range(B)), trace=False)
    y = np.stack([res.results[i]["y"] for i in range(B)])
    kernel.last_exec_time_ns = res.exec_time_ns
    return y
